# revision 46
# baseline (speedup 1.0000x reference)
"""Trainium2 Bass kernel for AlignGNN message passing (v2: PE-scatter).

Computation (per batch b):
    nh_fts = concat([node_fts, hidden], -1) @ W_nh + b_nh    # [N, M, OUT]
    coeff  = edge_fts @ W_e + b_e                            # [E, 1]
    out[n] = sum_{e: tgt[e]==n} coeff[e] * nh_fts[src[e]]    # [N, M, OUT]

Distribution: 8 cores = 4 batches x 2 edge-halves; host sums the two
partial outputs per batch.

Device pipeline per core:
  A) project nh_fts on TensorE (bf16 in, f32 accum), store bf16 rows
     [N, 256] to an HBM scratch.
  B) coeff = edge_fts @ W_e + b_e via DVE multiply+reduce (f32 accum).
  C) edges are host-sorted by target and grouped into aligned 128-node
     windows (padded to multiples of 128 with null edges). Per window:
     dma_gather the source rows (bf16, 4 SWDGE queues round-robin),
     scale host-encoded one-hot matrices S0[e, node] by the device
     coeff on DVE, and accumulate out_win = S^T @ V in PSUM on
     TensorE. Flush each window once to the f32 output.

DMA queues are split by phase: sync carries input streams + window
flushes, scalar(ACT HWDGE) carries nhf stores + S0 one-hot loads.

The dma_scatter_add primitive is NOT used for accumulation: its HBM
read-modify-write loses concurrent updates to duplicate rows on HW.

Host-side prep is layout-only: sorting/padding/permutation of the edge
stream, dtype casts, one-hot/descriptor encoding of the index tensors,
and replication of the small weights.
"""

import os

import numpy as np

import concourse.bass as bass
import concourse.bacc as bacc
import concourse.mybir as mybir
import concourse.tile as tile
from concourse.bass_utils import run_bass_kernel_spmd

# Problem sizes (fixed by the task)
B, N, E, M, H, OUT = 4, 10000, 160000, 4, 64, 64
F2 = 2 * H            # 128 contraction dim
NODE_ELEM = M * OUT   # 256 values per node row
NCORES = 8
EH = E // 2           # 80000 edges per core
NW = (N + 127) // 128  # 79 target windows
BLOCK = 2048          # stage-B coeff block
# gather group: must stay <= 1024 idxs/instruction — larger groups generate
# >128 SWDGE descriptors per direction and wedge the device (ring limit)
GCH = int(os.environ.get("KGCH", "1024"))
NQ = int(os.environ.get("KNQ", "4"))  # SWDGE queues for gather desc-gen
SP = os.environ.get("KSP", "0") == "1"  # gather single_packet mode
ROWTILE = 512
NROWT = (N * M) // ROWTILE        # 78 full row tiles
ROWREM = N * M - NROWT * ROWTILE  # 64

PAD_SHIFT = -512.0    # one-hot iota shift for padded edges (never matches)

_STATE: dict = {}

LAST_RESULT = None  # BassKernelResults of the most recent run (for test.py)


def _build_nc(caps):
    """caps: per-window padded edge counts (multiples of 128), same on all
    cores. Returns a compiled Bacc module."""
    stages = os.environ.get("KSTAGES", "full")  # debug bisect: A | AB | ABG | full
    T = int(sum(caps))
    assert T % 128 == 0
    TCOL = T // 128
    f32 = mybir.dt.float32
    bf16 = mybir.dt.bfloat16
    i16 = mybir.dt.int16
    nc = bacc.Bacc(
        None, target_bir_lowering=False, debug=False, num_swdge_queues=NQ
    )

    nht_d = nc.dram_tensor("nht", [F2, N * M], bf16, kind="ExternalInput")
    edge_d = nc.dram_tensor("edge", [T, H], bf16, kind="ExternalInput")
    srci_d = nc.dram_tensor("srci", [128, T // 16], i16, kind="ExternalInput")
    s0_d = nc.dram_tensor("s0", [128, TCOL * 128], bf16, kind="ExternalInput")
    wnh_d = nc.dram_tensor("wnh", [F2, OUT], bf16, kind="ExternalInput")
    bnh4_d = nc.dram_tensor("bnh4", [1, M * OUT], bf16, kind="ExternalInput")
    web_d = nc.dram_tensor("web", [128, H], bf16, kind="ExternalInput")
    beb_d = nc.dram_tensor("beb", [128, 1], f32, kind="ExternalInput")
    out_d = nc.dram_tensor("out", [N, NODE_ELEM], f32, kind="ExternalOutput")
    nhf_d = nc.dram_tensor("nhf", [N, NODE_ELEM], bf16)

    with tile.TileContext(nc) as tc:
        with (
            tc.tile_pool(name="const", bufs=1) as pc,
            tc.tile_pool(name="nht", bufs=3) as pa,
            tc.tile_pool(name="nhout", bufs=6) as pao,
            tc.tile_pool(name="edge", bufs=3) as pe,
            tc.tile_pool(name="vals", bufs=6) as pv,
            tc.tile_pool(name="onehot", bufs=6) as ps_pool,
            tc.tile_pool(name="s0", bufs=6) as ps0_pool,
            tc.tile_pool(name="flush", bufs=4) as pf,
            tc.tile_pool(name="psA", bufs=3, space="PSUM") as psA,
            tc.tile_pool(name="psW", bufs=4, space="PSUM") as psW,
        ):
            # --- constants ---
            wnh_t = pc.tile([F2, OUT], bf16)
            nc.sync.dma_start(wnh_t[:], wnh_d[:])
            bnh4_t = pc.tile([1, M * OUT], bf16)
            nc.sync.dma_start(bnh4_t[:], bnh4_d[:])
            web_t = pc.tile([128, H], bf16)
            nc.sync.dma_start(web_t[:], web_d[:])
            beb_t = pc.tile([128, 1], f32)
            nc.sync.dma_start(beb_t[:], beb_d[:])
            srci_t = pc.tile([128, T // 16], i16)
            nc.sync.dma_start(srci_t[:], srci_d[:])
            ones_t = pc.tile([1, 128], bf16)
            nc.gpsimd.memset(ones_t[:], 1.0)
            coeff_t = pc.tile([128, TCOL], f32)
            coefb_t = pc.tile([128, TCOL], bf16)
            # iota along free dim (same per partition, 4 copies)
            # PE warm-up: dependency-free matmuls so the HAM clock gate
            # releases (1.2 -> 2.4 GHz) before the real work arrives
            wrm = pc.tile([128, 512], bf16)
            nc.gpsimd.memset(wrm[:], 1.0)
            wps = psW.tile([128, 512], f32, tag="psW")
            for _ in range(16):
                nc.tensor.matmul(wps[:, :], wrm[:, :128], wrm[:], start=True, stop=True)

            # bias broadcast to all partitions via K=1 outer product
            bias_ps = psA.tile([128, M * OUT], f32, tag="psA")
            nc.tensor.matmul(bias_ps[:], ones_t[:], bnh4_t[:], start=True, stop=True)
            bias_t = pc.tile([128, M * OUT], f32)
            nc.any.tensor_copy(bias_t[:], bias_ps[:])

            # --- stage A: nh_fts projection -> nhf rows [N, 256] bf16 ---
            for t in range(NROWT + 1):
                cols = ROWTILE if t < NROWT else ROWREM
                if cols == 0:
                    break
                nodes = cols // M
                nht_t = pa.tile([F2, ROWTILE], bf16, tag="nht")
                nc.sync.dma_start(
                    nht_t[:, :cols], nht_d[:, t * ROWTILE : t * ROWTILE + cols]
                )
                ps = psA.tile([128, M, OUT], f32, tag="psA")
                for m in range(M):
                    lhsT = nht_t[:, m : cols : M]
                    nc.tensor.matmul(
                        ps[:nodes, m, :], lhsT, wnh_t[:], start=True, stop=True
                    )
                o_t = pao.tile([128, M * OUT], bf16, tag="nhout")
                nc.vector.tensor_tensor(
                    o_t[:nodes, :],
                    ps[:nodes, :, :].rearrange("p m o -> p (m o)"),
                    bias_t[:nodes, :],
                    op=mybir.AluOpType.add,
                )
                n0 = t * (ROWTILE // M)
                nc.scalar.dma_start(nhf_d[n0 : n0 + nodes, :], o_t[:nodes, :])

            # --- stage B: coeff for the padded sorted edge stream ---
            nblk = 0 if stages == "A" else (T + BLOCK - 1) // BLOCK
            for c in range(nblk):
                e0 = c * BLOCK
                ecnt = min(BLOCK, T - e0)
                j = ecnt // 128
                et = pe.tile([128, BLOCK // 128, H], bf16, tag="edge")
                nc.sync.dma_start(
                    et[:, :j, :],
                    edge_d[e0 : e0 + ecnt].rearrange("(p jj) f -> p jj f", p=128),
                )
                web_b = bass.AP(
                    web_t[:].tensor, web_t[:].offset,
                    [web_t[:].ap[0], [0, j], web_t[:].ap[1]],
                )
                nc.vector.tensor_tensor(
                    et[:, :j, :], et[:, :j, :], web_b, op=mybir.AluOpType.mult
                )
                ccol = e0 // 128
                csl = coeff_t[:, ccol : ccol + j]
                nc.vector.tensor_reduce(
                    csl, et[:, :j, :], axis=mybir.AxisListType.X,
                    op=mybir.AluOpType.add,
                )
                nc.vector.tensor_scalar(
                    csl, csl, beb_t[:], None, mybir.AluOpType.add
                )
                nc.vector.tensor_copy(coefb_t[:, ccol : ccol + j], csl)

            # --- stage C: gather + one-hot PE scatter per window ---
            # gather groups of GCH edges, round-robined over SWDGE queues
            vtiles = []
            ngr = 0 if stages in ("A", "AB") else (T + GCH - 1) // GCH
            for g in range(ngr):
                e0 = g * GCH
                ecnt = min(GCH, T - e0)
                v = pv.tile([128, GCH // 128, NODE_ELEM], bf16, tag="vals")
                nc.gpsimd.dma_gather(
                    v[:, : ecnt // 128, :], nhf_d[:],
                    srci_t[:, e0 // 16 : (e0 + ecnt) // 16],
                    num_idxs=ecnt, num_idxs_reg=ecnt, elem_size=NODE_ELEM,
                    queue_num=g % NQ, single_packet=SP,
                )
                vtiles.append(v)

            q = 0  # global 128-edge column
            for w in range(NW if stages == "full" else 0):
                nw = min(128, N - w * 128)
                nch = caps[w] // 128
                if nch == 0:
                    z_t = pf.tile([128, NODE_ELEM], f32, tag="flush")
                    nc.any.memset(z_t[:nw, :], 0.0)
                    nc.sync.dma_start(out_d[w * 128 : w * 128 + nw, :], z_t[:nw, :])
                    continue
                pw = psW.tile([128, NODE_ELEM], f32, tag="psW")
                k = 0
                while k < nch:
                    kb = min(8, nch - k)
                    S = ps_pool.tile([128, 8, 128], bf16, tag="S")
                    s0_t = ps0_pool.tile([128, 8, 128], bf16, tag="s0")
                    nc.scalar.dma_start(
                        s0_t[:, :kb, :],
                        s0_d[:, (q + k) * 128 : (q + k + kb) * 128].rearrange(
                            "p (kb f) -> p kb f", kb=kb
                        ),
                    )
                    cf_sl = coefb_t[:, q + k : q + k + kb]
                    cf_b = bass.AP(
                        cf_sl.tensor, cf_sl.offset,
                        [cf_sl.ap[0], cf_sl.ap[1], [0, 128]],
                    )
                    nc.vector.tensor_tensor(
                        S[:, :kb, :], s0_t[:, :kb, :], cf_b, op=mybir.AluOpType.mult
                    )
                    for kk in range(kb):
                        qq = q + k + kk
                        v = vtiles[qq // (GCH // 128)]
                        vcol = qq % (GCH // 128)
                        nc.tensor.matmul(
                            pw[:, :], S[:, kk, :], v[:, vcol, :],
                            start=(k + kk == 0), stop=(k + kk == nch - 1),
                        )
                    k += kb
                q += nch
                o_t = pf.tile([128, NODE_ELEM], f32, tag="flush")
                nc.any.tensor_copy(o_t[:nw, :], pw[:nw, :])
                nc.sync.dma_start(out_d[w * 128 : w * 128 + nw, :], o_t[:nw, :])

    nc.compile()
    return nc


def _wrap_idx(idx: np.ndarray) -> np.ndarray:
    """Wrap an index stream into the [16, n/16] descriptor layout (idx i at
    [i%16, i//16]) and replicate to 128 partitions."""
    w16 = idx.reshape(-1, 16).T
    return np.tile(w16, (8, 1)).astype(np.int16)


def _interleave_rows(a: np.ndarray) -> np.ndarray:
    """Reorder edge rows so a contiguous per-partition DMA of [128, j, F]
    tiles puts logical edge l = block*BLOCK + jj*128 + p at tile[p, jj].

    DRAM row for logical l must be block*BLOCK + p*j + jj."""
    T = a.shape[0]
    out = np.empty_like(a)
    e0 = 0
    while e0 < T:
        ecnt = min(BLOCK, T - e0)
        j = ecnt // 128
        blk = a[e0 : e0 + ecnt]          # logical order [jj*128+p]
        out[e0 : e0 + ecnt] = (
            blk.reshape(j, 128, -1).transpose(1, 0, 2).reshape(ecnt, -1)
        )
        e0 += ecnt
    return out


def _prep(node_fts, hidden, edge_fts, W_nh, b_nh, W_e, b_e, edge_indices):
    """Returns (caps, in_maps)."""
    # per-core sorted edge streams
    streams = []
    counts = np.zeros((NCORES, NW), np.int64)
    for c in range(NCORES):
        b, h = divmod(c, 2)
        sl = slice(h * EH, (h + 1) * EH)
        src = np.asarray(edge_indices[b, sl, 0], np.int64)
        tgt = np.asarray(edge_indices[b, sl, 1], np.int64)
        order = np.argsort(tgt, kind="stable")
        tgt_s = tgt[order]
        counts[c] = np.bincount(tgt_s // 128, minlength=NW)
        streams.append((b, sl, order, tgt_s, src[order]))
    caps = ((counts.max(axis=0) + 127) // 128) * 128
    T = int(caps.sum())
    caps = tuple(int(x) for x in caps)

    wnh = np.ascontiguousarray(W_nh).astype(np.float32).astype(mybir.dt.np(mybir.dt.bfloat16))
    bf = wnh.dtype
    bnh4 = np.tile(np.asarray(b_nh, np.float32).reshape(1, OUT), (1, M)).astype(bf)
    web = np.tile(np.asarray(W_e, np.float32).reshape(1, H), (128, 1)).astype(bf)
    beb = np.full((128, 1), np.float32(np.asarray(b_e).reshape(-1)[0]), np.float32)

    nht_b = {}
    in_maps = []
    wstart = np.zeros(NW + 1, np.int64)
    wstart[1:] = np.cumsum(caps)
    for c in range(NCORES):
        b, sl, order, tgt_s, src_s = streams[c]
        if b not in nht_b:
            a = np.empty((F2, N * M), np.float32)
            a[:H] = node_fts[b].reshape(-1, H).T
            a[H:] = hidden[b].reshape(-1, H).T
            nht_b[b] = a.astype(bf)
        # build padded streams
        src_p = np.zeros(T, np.int64)
        tshift = np.full(T, PAD_SHIFT, np.float32)
        epos = np.full(T, -1, np.int64)  # source row in edge_fts (unsorted), -1 pad
        cnt = counts[c]
        segs = np.zeros(NW + 1, np.int64)
        segs[1:] = np.cumsum(cnt)
        for w in range(NW):
            s0, s1 = segs[w], segs[w + 1]
            d0 = wstart[w]
            n = s1 - s0
            src_p[d0 : d0 + n] = src_s[s0:s1]
            tshift[d0 : d0 + n] = tgt_s[s0:s1] - 128 * w
            epos[d0 : d0 + n] = order[s0:s1]
        # edge features in padded-sorted order (pads zero), interleaved
        ef = np.zeros((T, H), np.float32)
        valid = epos >= 0
        ef[valid] = np.asarray(edge_fts[b, sl], np.float32)[epos[valid]]
        ef = _interleave_rows(ef.astype(bf))
        tsc = tshift.reshape(-1, 128).T  # [128, TCOL]
        s0u = np.zeros((128, tsc.shape[1] * 128), np.uint16)
        vmask = (tsc >= 0) & (tsc < 128)
        pp, qq = np.nonzero(vmask)
        s0u[pp, qq * 128 + tsc[pp, qq].astype(np.int64)] = 0x3F80
        in_maps.append({
            "nht": nht_b[b],
            "edge": ef,
            "srci": _wrap_idx(src_p.astype(np.int16)),
            "s0": s0u.view(bf),
            "wnh": wnh,
            "bnh4": bnh4,
            "web": web,
            "beb": beb,
        })
    return caps, in_maps


def _get_nc(caps):
    key = ("nc", caps)
    if key not in _STATE:
        _STATE[key] = _build_nc(caps)
    return _STATE[key]


def kernel(node_fts, hidden, edge_fts, W_nh, b_nh, W_e, b_e, edge_indices):
    global LAST_RESULT
    caps, in_maps = _prep(
        node_fts, hidden, edge_fts, W_nh, b_nh, W_e, b_e, edge_indices
    )
    nc = _get_nc(caps)
    res = run_bass_kernel_spmd(nc, in_maps, core_ids=list(range(NCORES)))
    LAST_RESULT = res
    out = np.empty((B, N, M, OUT), np.float32)
    for b in range(B):
        acc = res.results[2 * b]["out"] + res.results[2 * b + 1]["out"]
        out[b] = acc.reshape(N, M, OUT)
    return out


def run_core_sim(core_id, caps, in_map, zero_out=True):
    """Simulate a single core's program on its in_map via CoreSim (test use)."""
    from concourse.bass_interp import CoreSim

    nc = _get_nc(caps)
    sim = CoreSim(nc, trace=False)
    for k, v in in_map.items():
        sim.tensor(k)[:] = v
    if zero_out:
        sim.tensor("out")[:] = 0
    sim.simulate()
    return np.array(sim.tensor("out"))


# revision 47
# speedup vs baseline: 1.1905x; 1.1905x over previous
"""Trainium2 Bass kernel for AlignGNN message passing (v2: PE-scatter).

Computation (per batch b):
    nh_fts = concat([node_fts, hidden], -1) @ W_nh + b_nh    # [N, M, OUT]
    coeff  = edge_fts @ W_e + b_e                            # [E, 1]
    out[n] = sum_{e: tgt[e]==n} coeff[e] * nh_fts[src[e]]    # [N, M, OUT]

Distribution: 8 cores = 4 batches x 2 edge-halves; host sums the two
partial outputs per batch.

Device pipeline per core:
  A) project nh_fts on TensorE (bf16 in, f32 accum), store bf16 rows
     [N, 256] to an HBM scratch.
  B) coeff = edge_fts @ W_e + b_e via DVE multiply+reduce (f32 accum).
  C) edges are host-sorted by target and grouped into aligned 128-node
     windows (padded to multiples of 128 with null edges). Per window:
     dma_gather the source rows (bf16, 4 SWDGE queues round-robin),
     scale host-encoded one-hot matrices S0[e, node] by the device
     coeff on DVE, and accumulate out_win = S^T @ V in PSUM on
     TensorE. Flush each window once to the f32 output.

DMA queues are split by phase: sync carries input streams + window
flushes, scalar(ACT HWDGE) carries nhf stores + S0 one-hot loads.

The dma_scatter_add primitive is NOT used for accumulation: its HBM
read-modify-write loses concurrent updates to duplicate rows on HW.

Host-side prep is layout-only: sorting/padding/permutation of the edge
stream, dtype casts, one-hot/descriptor encoding of the index tensors,
and replication of the small weights.
"""

import os

import numpy as np

import concourse.bass as bass
import concourse.bacc as bacc
import concourse.mybir as mybir
import concourse.tile as tile
from concourse.bass_utils import run_bass_kernel_spmd

# Problem sizes (fixed by the task)
B, N, E, M, H, OUT = 4, 10000, 160000, 4, 64, 64
F2 = 2 * H            # 128 contraction dim
NODE_ELEM = M * OUT   # 256 values per node row
NCORES = 8
EH = E // 2           # 80000 edges per core
NW = (N + 127) // 128  # 79 target windows
BLOCK = 2048          # stage-B coeff block
# gather group: must stay <= 1024 idxs/instruction — larger groups generate
# >128 SWDGE descriptors per direction and wedge the device (ring limit)
GCH = int(os.environ.get("KGCH", "1024"))
NQ = int(os.environ.get("KNQ", "4"))  # SWDGE queues for gather desc-gen
SP = os.environ.get("KSP", "1") == "1"  # gather single_packet mode
ROWTILE = 512
NROWT = (N * M) // ROWTILE        # 78 full row tiles
ROWREM = N * M - NROWT * ROWTILE  # 64

PAD_SHIFT = -512.0    # one-hot iota shift for padded edges (never matches)

_STATE: dict = {}

LAST_RESULT = None  # BassKernelResults of the most recent run (for test.py)


def _build_nc(caps):
    """caps: per-window padded edge counts (multiples of 128), same on all
    cores. Returns a compiled Bacc module."""
    stages = os.environ.get("KSTAGES", "full")  # debug bisect: A | AB | ABG | full
    T = int(sum(caps))
    assert T % 128 == 0
    TCOL = T // 128
    f32 = mybir.dt.float32
    bf16 = mybir.dt.bfloat16
    i16 = mybir.dt.int16
    nc = bacc.Bacc(
        None, target_bir_lowering=False, debug=False, num_swdge_queues=NQ
    )

    nht_d = nc.dram_tensor("nht", [F2, N * M], bf16, kind="ExternalInput")
    edge_d = nc.dram_tensor("edge", [T, H], bf16, kind="ExternalInput")
    srci_d = nc.dram_tensor("srci", [128, T // 16], i16, kind="ExternalInput")
    s0_d = nc.dram_tensor("s0", [128, TCOL * 128], bf16, kind="ExternalInput")
    wnh_d = nc.dram_tensor("wnh", [F2, OUT], bf16, kind="ExternalInput")
    bnh4_d = nc.dram_tensor("bnh4", [1, M * OUT], bf16, kind="ExternalInput")
    web_d = nc.dram_tensor("web", [128, H], bf16, kind="ExternalInput")
    beb_d = nc.dram_tensor("beb", [128, 1], f32, kind="ExternalInput")
    out_d = nc.dram_tensor("out", [N, NODE_ELEM], f32, kind="ExternalOutput")
    nhf_d = nc.dram_tensor("nhf", [N, NODE_ELEM], bf16)

    with tile.TileContext(nc) as tc:
        with (
            tc.tile_pool(name="const", bufs=1) as pc,
            tc.tile_pool(name="nht", bufs=3) as pa,
            tc.tile_pool(name="nhout", bufs=6) as pao,
            tc.tile_pool(name="edge", bufs=3) as pe,
            tc.tile_pool(name="vals", bufs=6) as pv,
            tc.tile_pool(name="onehot", bufs=6) as ps_pool,
            tc.tile_pool(name="s0", bufs=6) as ps0_pool,
            tc.tile_pool(name="flush", bufs=4) as pf,
            tc.tile_pool(name="psA", bufs=3, space="PSUM") as psA,
            tc.tile_pool(name="psW", bufs=4, space="PSUM") as psW,
        ):
            # --- constants ---
            wnh_t = pc.tile([F2, OUT], bf16)
            nc.sync.dma_start(wnh_t[:], wnh_d[:])
            bnh4_t = pc.tile([1, M * OUT], bf16)
            nc.sync.dma_start(bnh4_t[:], bnh4_d[:])
            web_t = pc.tile([128, H], bf16)
            nc.sync.dma_start(web_t[:], web_d[:])
            beb_t = pc.tile([128, 1], f32)
            nc.sync.dma_start(beb_t[:], beb_d[:])
            srci_t = pc.tile([128, T // 16], i16)
            nc.sync.dma_start(srci_t[:], srci_d[:])
            ones_t = pc.tile([1, 128], bf16)
            nc.gpsimd.memset(ones_t[:], 1.0)
            coeff_t = pc.tile([128, TCOL], f32)
            coefb_t = pc.tile([128, TCOL], bf16)
            # iota along free dim (same per partition, 4 copies)
            # PE warm-up: dependency-free matmuls so the HAM clock gate
            # releases (1.2 -> 2.4 GHz) before the real work arrives
            wrm = pc.tile([128, 512], bf16)
            nc.gpsimd.memset(wrm[:], 1.0)
            wps = psW.tile([128, 512], f32, tag="psW")
            for _ in range(16):
                nc.tensor.matmul(wps[:, :], wrm[:, :128], wrm[:], start=True, stop=True)

            # bias broadcast to all partitions via K=1 outer product
            bias_ps = psA.tile([128, M * OUT], f32, tag="psA")
            nc.tensor.matmul(bias_ps[:], ones_t[:], bnh4_t[:], start=True, stop=True)
            bias_t = pc.tile([128, M * OUT], f32)
            nc.any.tensor_copy(bias_t[:], bias_ps[:])

            # --- stage A: nh_fts projection -> nhf rows [N, 256] bf16 ---
            for t in range(NROWT + 1):
                cols = ROWTILE if t < NROWT else ROWREM
                if cols == 0:
                    break
                nodes = cols // M
                nht_t = pa.tile([F2, ROWTILE], bf16, tag="nht")
                nc.sync.dma_start(
                    nht_t[:, :cols], nht_d[:, t * ROWTILE : t * ROWTILE + cols]
                )
                ps = psA.tile([128, M, OUT], f32, tag="psA")
                for m in range(M):
                    lhsT = nht_t[:, m : cols : M]
                    nc.tensor.matmul(
                        ps[:nodes, m, :], lhsT, wnh_t[:], start=True, stop=True
                    )
                o_t = pao.tile([128, M * OUT], bf16, tag="nhout")
                nc.vector.tensor_tensor(
                    o_t[:nodes, :],
                    ps[:nodes, :, :].rearrange("p m o -> p (m o)"),
                    bias_t[:nodes, :],
                    op=mybir.AluOpType.add,
                )
                n0 = t * (ROWTILE // M)
                nc.scalar.dma_start(nhf_d[n0 : n0 + nodes, :], o_t[:nodes, :])

            # --- stage B: coeff for the padded sorted edge stream ---
            nblk = 0 if stages == "A" else (T + BLOCK - 1) // BLOCK
            for c in range(nblk):
                e0 = c * BLOCK
                ecnt = min(BLOCK, T - e0)
                j = ecnt // 128
                et = pe.tile([128, BLOCK // 128, H], bf16, tag="edge")
                nc.sync.dma_start(
                    et[:, :j, :],
                    edge_d[e0 : e0 + ecnt].rearrange("(p jj) f -> p jj f", p=128),
                )
                web_b = bass.AP(
                    web_t[:].tensor, web_t[:].offset,
                    [web_t[:].ap[0], [0, j], web_t[:].ap[1]],
                )
                nc.vector.tensor_tensor(
                    et[:, :j, :], et[:, :j, :], web_b, op=mybir.AluOpType.mult
                )
                ccol = e0 // 128
                csl = coeff_t[:, ccol : ccol + j]
                nc.vector.tensor_reduce(
                    csl, et[:, :j, :], axis=mybir.AxisListType.X,
                    op=mybir.AluOpType.add,
                )
                nc.vector.tensor_scalar(
                    csl, csl, beb_t[:], None, mybir.AluOpType.add
                )
                nc.vector.tensor_copy(coefb_t[:, ccol : ccol + j], csl)

            # --- stage C: gather + one-hot PE scatter per window ---
            # gather groups of GCH edges, round-robined over SWDGE queues
            vtiles = []
            ngr = 0 if stages in ("A", "AB") else (T + GCH - 1) // GCH
            for g in range(ngr):
                e0 = g * GCH
                ecnt = min(GCH, T - e0)
                v = pv.tile([128, GCH // 128, NODE_ELEM], bf16, tag="vals")
                nc.gpsimd.dma_gather(
                    v[:, : ecnt // 128, :], nhf_d[:],
                    srci_t[:, e0 // 16 : (e0 + ecnt) // 16],
                    num_idxs=ecnt, num_idxs_reg=ecnt, elem_size=NODE_ELEM,
                    queue_num=g % NQ, single_packet=SP,
                )
                vtiles.append(v)

            q = 0  # global 128-edge column
            for w in range(NW if stages == "full" else 0):
                nw = min(128, N - w * 128)
                nch = caps[w] // 128
                if nch == 0:
                    z_t = pf.tile([128, NODE_ELEM], f32, tag="flush")
                    nc.any.memset(z_t[:nw, :], 0.0)
                    nc.sync.dma_start(out_d[w * 128 : w * 128 + nw, :], z_t[:nw, :])
                    continue
                pw = psW.tile([128, NODE_ELEM], f32, tag="psW")
                k = 0
                while k < nch:
                    kb = min(8, nch - k)
                    S = ps_pool.tile([128, 8, 128], bf16, tag="S")
                    s0_t = ps0_pool.tile([128, 8, 128], bf16, tag="s0")
                    nc.scalar.dma_start(
                        s0_t[:, :kb, :],
                        s0_d[:, (q + k) * 128 : (q + k + kb) * 128].rearrange(
                            "p (kb f) -> p kb f", kb=kb
                        ),
                    )
                    cf_sl = coefb_t[:, q + k : q + k + kb]
                    cf_b = bass.AP(
                        cf_sl.tensor, cf_sl.offset,
                        [cf_sl.ap[0], cf_sl.ap[1], [0, 128]],
                    )
                    nc.vector.tensor_tensor(
                        S[:, :kb, :], s0_t[:, :kb, :], cf_b, op=mybir.AluOpType.mult
                    )
                    for kk in range(kb):
                        qq = q + k + kk
                        v = vtiles[qq // (GCH // 128)]
                        vcol = qq % (GCH // 128)
                        nc.tensor.matmul(
                            pw[:, :], S[:, kk, :], v[:, vcol, :],
                            start=(k + kk == 0), stop=(k + kk == nch - 1),
                        )
                    k += kb
                q += nch
                o_t = pf.tile([128, NODE_ELEM], f32, tag="flush")
                nc.any.tensor_copy(o_t[:nw, :], pw[:nw, :])
                nc.sync.dma_start(out_d[w * 128 : w * 128 + nw, :], o_t[:nw, :])

    nc.compile()
    return nc


def _wrap_idx(idx: np.ndarray) -> np.ndarray:
    """Wrap an index stream into the [16, n/16] descriptor layout (idx i at
    [i%16, i//16]) and replicate to 128 partitions."""
    w16 = idx.reshape(-1, 16).T
    return np.tile(w16, (8, 1)).astype(np.int16)


def _interleave_rows(a: np.ndarray) -> np.ndarray:
    """Reorder edge rows so a contiguous per-partition DMA of [128, j, F]
    tiles puts logical edge l = block*BLOCK + jj*128 + p at tile[p, jj].

    DRAM row for logical l must be block*BLOCK + p*j + jj."""
    T = a.shape[0]
    out = np.empty_like(a)
    e0 = 0
    while e0 < T:
        ecnt = min(BLOCK, T - e0)
        j = ecnt // 128
        blk = a[e0 : e0 + ecnt]          # logical order [jj*128+p]
        out[e0 : e0 + ecnt] = (
            blk.reshape(j, 128, -1).transpose(1, 0, 2).reshape(ecnt, -1)
        )
        e0 += ecnt
    return out


def _prep(node_fts, hidden, edge_fts, W_nh, b_nh, W_e, b_e, edge_indices):
    """Returns (caps, in_maps)."""
    # per-core sorted edge streams
    streams = []
    counts = np.zeros((NCORES, NW), np.int64)
    for c in range(NCORES):
        b, h = divmod(c, 2)
        sl = slice(h * EH, (h + 1) * EH)
        src = np.asarray(edge_indices[b, sl, 0], np.int64)
        tgt = np.asarray(edge_indices[b, sl, 1], np.int64)
        order = np.argsort(tgt, kind="stable")
        tgt_s = tgt[order]
        counts[c] = np.bincount(tgt_s // 128, minlength=NW)
        streams.append((b, sl, order, tgt_s, src[order]))
    caps = ((counts.max(axis=0) + 127) // 128) * 128
    T = int(caps.sum())
    caps = tuple(int(x) for x in caps)

    wnh = np.ascontiguousarray(W_nh).astype(np.float32).astype(mybir.dt.np(mybir.dt.bfloat16))
    bf = wnh.dtype
    bnh4 = np.tile(np.asarray(b_nh, np.float32).reshape(1, OUT), (1, M)).astype(bf)
    web = np.tile(np.asarray(W_e, np.float32).reshape(1, H), (128, 1)).astype(bf)
    beb = np.full((128, 1), np.float32(np.asarray(b_e).reshape(-1)[0]), np.float32)

    nht_b = {}
    in_maps = []
    wstart = np.zeros(NW + 1, np.int64)
    wstart[1:] = np.cumsum(caps)
    for c in range(NCORES):
        b, sl, order, tgt_s, src_s = streams[c]
        if b not in nht_b:
            a = np.empty((F2, N * M), np.float32)
            a[:H] = node_fts[b].reshape(-1, H).T
            a[H:] = hidden[b].reshape(-1, H).T
            nht_b[b] = a.astype(bf)
        # build padded streams
        src_p = np.zeros(T, np.int64)
        tshift = np.full(T, PAD_SHIFT, np.float32)
        epos = np.full(T, -1, np.int64)  # source row in edge_fts (unsorted), -1 pad
        cnt = counts[c]
        segs = np.zeros(NW + 1, np.int64)
        segs[1:] = np.cumsum(cnt)
        for w in range(NW):
            s0, s1 = segs[w], segs[w + 1]
            d0 = wstart[w]
            n = s1 - s0
            src_p[d0 : d0 + n] = src_s[s0:s1]
            tshift[d0 : d0 + n] = tgt_s[s0:s1] - 128 * w
            epos[d0 : d0 + n] = order[s0:s1]
        # edge features in padded-sorted order (pads zero), interleaved
        ef = np.zeros((T, H), np.float32)
        valid = epos >= 0
        ef[valid] = np.asarray(edge_fts[b, sl], np.float32)[epos[valid]]
        ef = _interleave_rows(ef.astype(bf))
        tsc = tshift.reshape(-1, 128).T  # [128, TCOL]
        s0u = np.zeros((128, tsc.shape[1] * 128), np.uint16)
        vmask = (tsc >= 0) & (tsc < 128)
        pp, qq = np.nonzero(vmask)
        s0u[pp, qq * 128 + tsc[pp, qq].astype(np.int64)] = 0x3F80
        in_maps.append({
            "nht": nht_b[b],
            "edge": ef,
            "srci": _wrap_idx(src_p.astype(np.int16)),
            "s0": s0u.view(bf),
            "wnh": wnh,
            "bnh4": bnh4,
            "web": web,
            "beb": beb,
        })
    return caps, in_maps


def _get_nc(caps):
    key = ("nc", caps)
    if key not in _STATE:
        _STATE[key] = _build_nc(caps)
    return _STATE[key]


def kernel(node_fts, hidden, edge_fts, W_nh, b_nh, W_e, b_e, edge_indices):
    global LAST_RESULT
    caps, in_maps = _prep(
        node_fts, hidden, edge_fts, W_nh, b_nh, W_e, b_e, edge_indices
    )
    nc = _get_nc(caps)
    res = run_bass_kernel_spmd(nc, in_maps, core_ids=list(range(NCORES)))
    LAST_RESULT = res
    out = np.empty((B, N, M, OUT), np.float32)
    for b in range(B):
        acc = res.results[2 * b]["out"] + res.results[2 * b + 1]["out"]
        out[b] = acc.reshape(N, M, OUT)
    return out


def run_core_sim(core_id, caps, in_map, zero_out=True):
    """Simulate a single core's program on its in_map via CoreSim (test use)."""
    from concourse.bass_interp import CoreSim

    nc = _get_nc(caps)
    sim = CoreSim(nc, trace=False)
    for k, v in in_map.items():
        sim.tensor(k)[:] = v
    if zero_out:
        sim.tensor("out")[:] = 0
    sim.simulate()
    return np.array(sim.tensor("out"))


# revision 48
# speedup vs baseline: 1.2369x; 1.0389x over previous
"""Trainium2 Bass kernel for AlignGNN message passing (v2: PE-scatter).

Computation (per batch b):
    nh_fts = concat([node_fts, hidden], -1) @ W_nh + b_nh    # [N, M, OUT]
    coeff  = edge_fts @ W_e + b_e                            # [E, 1]
    out[n] = sum_{e: tgt[e]==n} coeff[e] * nh_fts[src[e]]    # [N, M, OUT]

Distribution: 8 cores = 4 batches x 2 edge-halves; host sums the two
partial outputs per batch.

Device pipeline per core:
  A) project nh_fts on TensorE (bf16 in, f32 accum), store bf16 rows
     [N, 256] to an HBM scratch.
  B) coeff = edge_fts @ W_e + b_e via DVE multiply+reduce (f32 accum).
  C) edges are host-sorted by target and grouped into aligned 128-node
     windows (padded to multiples of 128 with null edges). Per window:
     dma_gather the source rows (bf16, 4 SWDGE queues round-robin),
     scale host-encoded one-hot matrices S0[e, node] by the device
     coeff on DVE, and accumulate out_win = S^T @ V in PSUM on
     TensorE. Flush each window once to the f32 output.

DMA queues are split by phase: sync carries input streams + window
flushes, scalar(ACT HWDGE) carries nhf stores + S0 one-hot loads.

The dma_scatter_add primitive is NOT used for accumulation: its HBM
read-modify-write loses concurrent updates to duplicate rows on HW.

Host-side prep is layout-only: sorting/padding/permutation of the edge
stream, dtype casts, one-hot/descriptor encoding of the index tensors,
and replication of the small weights.
"""

import os

import numpy as np

import concourse.bass as bass
import concourse.bacc as bacc
import concourse.mybir as mybir
import concourse.tile as tile
from concourse.bass_utils import run_bass_kernel_spmd

# Problem sizes (fixed by the task)
B, N, E, M, H, OUT = 4, 10000, 160000, 4, 64, 64
F2 = 2 * H            # 128 contraction dim
NODE_ELEM = M * OUT   # 256 values per node row
NCORES = 8
EH = E // 2           # 80000 edges per core
NW = (N + 127) // 128  # 79 target windows
BLOCK = 4096          # stage-B coeff block
# gather group: must stay <= 1024 idxs/instruction — larger groups generate
# >128 SWDGE descriptors per direction and wedge the device (ring limit)
GCH = int(os.environ.get("KGCH", "1024"))
NQ = int(os.environ.get("KNQ", "4"))  # SWDGE queues for gather desc-gen
SP = os.environ.get("KSP", "1") == "1"  # gather single_packet mode
ROWTILE = 1024
NROWT = (N * M) // ROWTILE        # 78 full row tiles
ROWREM = N * M - NROWT * ROWTILE  # 64

PAD_SHIFT = -512.0    # one-hot iota shift for padded edges (never matches)

_STATE: dict = {}

LAST_RESULT = None  # BassKernelResults of the most recent run (for test.py)


def _build_nc(caps):
    """caps: per-window padded edge counts (multiples of 128), same on all
    cores. Returns a compiled Bacc module."""
    stages = os.environ.get("KSTAGES", "full")  # debug bisect: A | AB | ABG | full
    T = int(sum(caps))
    assert T % 128 == 0
    TCOL = T // 128
    f32 = mybir.dt.float32
    bf16 = mybir.dt.bfloat16
    i16 = mybir.dt.int16
    nc = bacc.Bacc(
        None, target_bir_lowering=False, debug=False, num_swdge_queues=NQ
    )

    nht_d = nc.dram_tensor("nht", [F2, N * M], bf16, kind="ExternalInput")
    edge_d = nc.dram_tensor("edge", [T, H], bf16, kind="ExternalInput")
    srci_d = nc.dram_tensor("srci", [128, T // 16], i16, kind="ExternalInput")
    s0_d = nc.dram_tensor("s0", [128, TCOL * 128], bf16, kind="ExternalInput")
    wnh_d = nc.dram_tensor("wnh", [F2, OUT], bf16, kind="ExternalInput")
    bnh4_d = nc.dram_tensor("bnh4", [1, M * OUT], bf16, kind="ExternalInput")
    web_d = nc.dram_tensor("web", [128, H], bf16, kind="ExternalInput")
    beb_d = nc.dram_tensor("beb", [128, 1], f32, kind="ExternalInput")
    out_d = nc.dram_tensor("out", [N, NODE_ELEM], f32, kind="ExternalOutput")
    nhf_d = nc.dram_tensor("nhf", [N, NODE_ELEM], bf16)

    with tile.TileContext(nc) as tc:
        with (
            tc.tile_pool(name="const", bufs=1) as pc,
            tc.tile_pool(name="nht", bufs=3) as pa,
            tc.tile_pool(name="nhout", bufs=6) as pao,
            tc.tile_pool(name="edge", bufs=3) as pe,
            tc.tile_pool(name="vals", bufs=6) as pv,
            tc.tile_pool(name="onehot", bufs=6) as ps_pool,
            tc.tile_pool(name="s0", bufs=6) as ps0_pool,
            tc.tile_pool(name="flush", bufs=4) as pf,
            tc.tile_pool(name="psA", bufs=3, space="PSUM") as psA,
            tc.tile_pool(name="psW", bufs=4, space="PSUM") as psW,
        ):
            # --- constants ---
            wnh_t = pc.tile([F2, OUT], bf16)
            nc.sync.dma_start(wnh_t[:], wnh_d[:])
            bnh4_t = pc.tile([1, M * OUT], bf16)
            nc.sync.dma_start(bnh4_t[:], bnh4_d[:])
            web_t = pc.tile([128, H], bf16)
            nc.sync.dma_start(web_t[:], web_d[:])
            beb_t = pc.tile([128, 1], f32)
            nc.sync.dma_start(beb_t[:], beb_d[:])
            srci_t = pc.tile([128, T // 16], i16)
            nc.sync.dma_start(srci_t[:], srci_d[:])
            ones_t = pc.tile([1, 128], bf16)
            nc.gpsimd.memset(ones_t[:], 1.0)
            coeff_t = pc.tile([128, TCOL], f32)
            coefb_t = pc.tile([128, TCOL], bf16)
            # iota along free dim (same per partition, 4 copies)
            # PE warm-up: dependency-free matmuls so the HAM clock gate
            # releases (1.2 -> 2.4 GHz) before the real work arrives
            wrm = pc.tile([128, 512], bf16)
            nc.gpsimd.memset(wrm[:], 1.0)
            wps = psW.tile([128, 512], f32, tag="psW")
            for _ in range(16):
                nc.tensor.matmul(wps[:, :], wrm[:, :128], wrm[:], start=True, stop=True)

            # bias broadcast to all partitions via K=1 outer product
            bias_ps = psA.tile([128, M * OUT], f32, tag="psA")
            nc.tensor.matmul(bias_ps[:], ones_t[:], bnh4_t[:], start=True, stop=True)
            bias_t = pc.tile([128, M * OUT], f32)
            nc.any.tensor_copy(bias_t[:], bias_ps[:])

            # --- stage A: nh_fts projection -> nhf rows [N, 256] bf16 ---
            for t in range(NROWT + 1):
                cols = ROWTILE if t < NROWT else ROWREM
                if cols == 0:
                    break
                nht_t = pa.tile([F2, ROWTILE], bf16, tag="nht")
                nc.sync.dma_start(
                    nht_t[:, :cols], nht_d[:, t * ROWTILE : t * ROWTILE + cols]
                )
                for hh in range(0, cols, 512):
                    sub = min(512, cols - hh)
                    nodes = sub // M
                    ps = psA.tile([128, M, OUT], f32, tag="psA")
                    for m in range(M):
                        lhsT = nht_t[:, hh + m : hh + sub : M]
                        nc.tensor.matmul(
                            ps[:nodes, m, :], lhsT, wnh_t[:], start=True, stop=True
                        )
                    o_t = pao.tile([128, M * OUT], bf16, tag="nhout")
                    nc.vector.tensor_tensor(
                        o_t[:nodes, :],
                        ps[:nodes, :, :].rearrange("p m o -> p (m o)"),
                        bias_t[:nodes, :],
                        op=mybir.AluOpType.add,
                    )
                    n0 = (t * ROWTILE + hh) // M
                    nc.scalar.dma_start(nhf_d[n0 : n0 + nodes, :], o_t[:nodes, :])

            # --- stage B: coeff for the padded sorted edge stream ---
            nblk = 0 if stages == "A" else (T + BLOCK - 1) // BLOCK
            for c in range(nblk):
                e0 = c * BLOCK
                ecnt = min(BLOCK, T - e0)
                j = ecnt // 128
                et = pe.tile([128, BLOCK // 128, H], bf16, tag="edge")
                nc.sync.dma_start(
                    et[:, :j, :],
                    edge_d[e0 : e0 + ecnt].rearrange("(p jj) f -> p jj f", p=128),
                )
                web_b = bass.AP(
                    web_t[:].tensor, web_t[:].offset,
                    [web_t[:].ap[0], [0, j], web_t[:].ap[1]],
                )
                nc.vector.tensor_tensor(
                    et[:, :j, :], et[:, :j, :], web_b, op=mybir.AluOpType.mult
                )
                ccol = e0 // 128
                csl = coeff_t[:, ccol : ccol + j]
                nc.vector.tensor_reduce(
                    csl, et[:, :j, :], axis=mybir.AxisListType.X,
                    op=mybir.AluOpType.add,
                )
                nc.vector.tensor_scalar(
                    csl, csl, beb_t[:], None, mybir.AluOpType.add
                )
                nc.vector.tensor_copy(coefb_t[:, ccol : ccol + j], csl)

            # --- stage C: gather + one-hot PE scatter per window ---
            # gather groups of GCH edges, round-robined over SWDGE queues
            vtiles = []
            ngr = 0 if stages in ("A", "AB") else (T + GCH - 1) // GCH
            for g in range(ngr):
                e0 = g * GCH
                ecnt = min(GCH, T - e0)
                v = pv.tile([128, GCH // 128, NODE_ELEM], bf16, tag="vals")
                nc.gpsimd.dma_gather(
                    v[:, : ecnt // 128, :], nhf_d[:],
                    srci_t[:, e0 // 16 : (e0 + ecnt) // 16],
                    num_idxs=ecnt, num_idxs_reg=ecnt, elem_size=NODE_ELEM,
                    queue_num=g % NQ, single_packet=SP,
                )
                vtiles.append(v)

            q = 0  # global 128-edge column
            for w in range(NW if stages == "full" else 0):
                nw = min(128, N - w * 128)
                nch = caps[w] // 128
                if nch == 0:
                    z_t = pf.tile([128, NODE_ELEM], f32, tag="flush")
                    nc.any.memset(z_t[:nw, :], 0.0)
                    nc.sync.dma_start(out_d[w * 128 : w * 128 + nw, :], z_t[:nw, :])
                    continue
                pw = psW.tile([128, NODE_ELEM], f32, tag="psW")
                k = 0
                while k < nch:
                    kb = min(8, nch - k)
                    S = ps_pool.tile([128, 8, 128], bf16, tag="S")
                    s0_t = ps0_pool.tile([128, 8, 128], bf16, tag="s0")
                    nc.scalar.dma_start(
                        s0_t[:, :kb, :],
                        s0_d[:, (q + k) * 128 : (q + k + kb) * 128].rearrange(
                            "p (kb f) -> p kb f", kb=kb
                        ),
                    )
                    cf_sl = coefb_t[:, q + k : q + k + kb]
                    cf_b = bass.AP(
                        cf_sl.tensor, cf_sl.offset,
                        [cf_sl.ap[0], cf_sl.ap[1], [0, 128]],
                    )
                    nc.vector.tensor_tensor(
                        S[:, :kb, :], s0_t[:, :kb, :], cf_b, op=mybir.AluOpType.mult
                    )
                    for kk in range(kb):
                        qq = q + k + kk
                        v = vtiles[qq // (GCH // 128)]
                        vcol = qq % (GCH // 128)
                        nc.tensor.matmul(
                            pw[:, :], S[:, kk, :], v[:, vcol, :],
                            start=(k + kk == 0), stop=(k + kk == nch - 1),
                        )
                    k += kb
                q += nch
                o_t = pf.tile([128, NODE_ELEM], f32, tag="flush")
                nc.any.tensor_copy(o_t[:nw, :], pw[:nw, :])
                nc.sync.dma_start(out_d[w * 128 : w * 128 + nw, :], o_t[:nw, :])

    nc.compile()
    return nc


def _wrap_idx(idx: np.ndarray) -> np.ndarray:
    """Wrap an index stream into the [16, n/16] descriptor layout (idx i at
    [i%16, i//16]) and replicate to 128 partitions."""
    w16 = idx.reshape(-1, 16).T
    return np.tile(w16, (8, 1)).astype(np.int16)


def _interleave_rows(a: np.ndarray) -> np.ndarray:
    """Reorder edge rows so a contiguous per-partition DMA of [128, j, F]
    tiles puts logical edge l = block*BLOCK + jj*128 + p at tile[p, jj].

    DRAM row for logical l must be block*BLOCK + p*j + jj."""
    T = a.shape[0]
    out = np.empty_like(a)
    e0 = 0
    while e0 < T:
        ecnt = min(BLOCK, T - e0)
        j = ecnt // 128
        blk = a[e0 : e0 + ecnt]          # logical order [jj*128+p]
        out[e0 : e0 + ecnt] = (
            blk.reshape(j, 128, -1).transpose(1, 0, 2).reshape(ecnt, -1)
        )
        e0 += ecnt
    return out


def _prep(node_fts, hidden, edge_fts, W_nh, b_nh, W_e, b_e, edge_indices):
    """Returns (caps, in_maps)."""
    # per-core sorted edge streams
    streams = []
    counts = np.zeros((NCORES, NW), np.int64)
    for c in range(NCORES):
        b, h = divmod(c, 2)
        sl = slice(h * EH, (h + 1) * EH)
        src = np.asarray(edge_indices[b, sl, 0], np.int64)
        tgt = np.asarray(edge_indices[b, sl, 1], np.int64)
        order = np.argsort(tgt, kind="stable")
        tgt_s = tgt[order]
        counts[c] = np.bincount(tgt_s // 128, minlength=NW)
        streams.append((b, sl, order, tgt_s, src[order]))
    caps = ((counts.max(axis=0) + 127) // 128) * 128
    T = int(caps.sum())
    caps = tuple(int(x) for x in caps)

    wnh = np.ascontiguousarray(W_nh).astype(np.float32).astype(mybir.dt.np(mybir.dt.bfloat16))
    bf = wnh.dtype
    bnh4 = np.tile(np.asarray(b_nh, np.float32).reshape(1, OUT), (1, M)).astype(bf)
    web = np.tile(np.asarray(W_e, np.float32).reshape(1, H), (128, 1)).astype(bf)
    beb = np.full((128, 1), np.float32(np.asarray(b_e).reshape(-1)[0]), np.float32)

    nht_b = {}
    in_maps = []
    wstart = np.zeros(NW + 1, np.int64)
    wstart[1:] = np.cumsum(caps)
    for c in range(NCORES):
        b, sl, order, tgt_s, src_s = streams[c]
        if b not in nht_b:
            a = np.empty((F2, N * M), np.float32)
            a[:H] = node_fts[b].reshape(-1, H).T
            a[H:] = hidden[b].reshape(-1, H).T
            nht_b[b] = a.astype(bf)
        # build padded streams
        src_p = np.zeros(T, np.int64)
        tshift = np.full(T, PAD_SHIFT, np.float32)
        epos = np.full(T, -1, np.int64)  # source row in edge_fts (unsorted), -1 pad
        cnt = counts[c]
        segs = np.zeros(NW + 1, np.int64)
        segs[1:] = np.cumsum(cnt)
        for w in range(NW):
            s0, s1 = segs[w], segs[w + 1]
            d0 = wstart[w]
            n = s1 - s0
            src_p[d0 : d0 + n] = src_s[s0:s1]
            tshift[d0 : d0 + n] = tgt_s[s0:s1] - 128 * w
            epos[d0 : d0 + n] = order[s0:s1]
        # edge features in padded-sorted order (pads zero), interleaved
        ef = np.zeros((T, H), np.float32)
        valid = epos >= 0
        ef[valid] = np.asarray(edge_fts[b, sl], np.float32)[epos[valid]]
        ef = _interleave_rows(ef.astype(bf))
        tsc = tshift.reshape(-1, 128).T  # [128, TCOL]
        s0u = np.zeros((128, tsc.shape[1] * 128), np.uint16)
        vmask = (tsc >= 0) & (tsc < 128)
        pp, qq = np.nonzero(vmask)
        s0u[pp, qq * 128 + tsc[pp, qq].astype(np.int64)] = 0x3F80
        in_maps.append({
            "nht": nht_b[b],
            "edge": ef,
            "srci": _wrap_idx(src_p.astype(np.int16)),
            "s0": s0u.view(bf),
            "wnh": wnh,
            "bnh4": bnh4,
            "web": web,
            "beb": beb,
        })
    return caps, in_maps


def _get_nc(caps):
    key = ("nc", caps)
    if key not in _STATE:
        _STATE[key] = _build_nc(caps)
    return _STATE[key]


def kernel(node_fts, hidden, edge_fts, W_nh, b_nh, W_e, b_e, edge_indices):
    global LAST_RESULT
    caps, in_maps = _prep(
        node_fts, hidden, edge_fts, W_nh, b_nh, W_e, b_e, edge_indices
    )
    nc = _get_nc(caps)
    res = run_bass_kernel_spmd(nc, in_maps, core_ids=list(range(NCORES)))
    LAST_RESULT = res
    out = np.empty((B, N, M, OUT), np.float32)
    for b in range(B):
        acc = res.results[2 * b]["out"] + res.results[2 * b + 1]["out"]
        out[b] = acc.reshape(N, M, OUT)
    return out


def run_core_sim(core_id, caps, in_map, zero_out=True):
    """Simulate a single core's program on its in_map via CoreSim (test use)."""
    from concourse.bass_interp import CoreSim

    nc = _get_nc(caps)
    sim = CoreSim(nc, trace=False)
    for k, v in in_map.items():
        sim.tensor(k)[:] = v
    if zero_out:
        sim.tensor("out")[:] = 0
    sim.simulate()
    return np.array(sim.tensor("out"))


# revision 49
# speedup vs baseline: 1.2930x; 1.0454x over previous
"""Trainium2 Bass kernel for AlignGNN message passing (v2: PE-scatter).

Computation (per batch b):
    nh_fts = concat([node_fts, hidden], -1) @ W_nh + b_nh    # [N, M, OUT]
    coeff  = edge_fts @ W_e + b_e                            # [E, 1]
    out[n] = sum_{e: tgt[e]==n} coeff[e] * nh_fts[src[e]]    # [N, M, OUT]

Distribution: 8 cores = 4 batches x 2 edge-halves; host sums the two
partial outputs per batch.

Device pipeline per core:
  A) project nh_fts on TensorE (bf16 in, f32 accum), store bf16 rows
     [N, 256] to an HBM scratch.
  B) coeff = edge_fts @ W_e + b_e via DVE multiply+reduce (f32 accum).
  C) edges are host-sorted by target and grouped into aligned 128-node
     windows (padded to multiples of 128 with null edges). Per window:
     dma_gather the source rows (bf16, 4 SWDGE queues round-robin),
     scale host-encoded one-hot matrices S0[e, node] by the device
     coeff on DVE, and accumulate out_win = S^T @ V in PSUM on
     TensorE. Flush each window once to the f32 output.

DMA queues are split by phase: sync carries input streams + window
flushes, scalar(ACT HWDGE) carries nhf stores + S0 one-hot loads.

The dma_scatter_add primitive is NOT used for accumulation: its HBM
read-modify-write loses concurrent updates to duplicate rows on HW.

Host-side prep is layout-only: sorting/padding/permutation of the edge
stream, dtype casts, one-hot/descriptor encoding of the index tensors,
and replication of the small weights.
"""

import os

import numpy as np

import concourse.bass as bass
import concourse.bacc as bacc
import concourse.mybir as mybir
import concourse.tile as tile
from concourse.bass_utils import run_bass_kernel_spmd

# Problem sizes (fixed by the task)
B, N, E, M, H, OUT = 4, 10000, 160000, 4, 64, 64
F2 = 2 * H            # 128 contraction dim
NODE_ELEM = M * OUT   # 256 values per node row
NCORES = 8
EH = E // 2           # 80000 edges per core
NW = (N + 127) // 128  # 79 target windows
BLOCK = 8192          # stage-B coeff block
# gather group: must stay <= 1024 idxs/instruction — larger groups generate
# >128 SWDGE descriptors per direction and wedge the device (ring limit)
GCH = int(os.environ.get("KGCH", "1024"))
NQ = int(os.environ.get("KNQ", "4"))  # SWDGE queues for gather desc-gen
SP = os.environ.get("KSP", "1") == "1"  # gather single_packet mode
ROWTILE = 2048
NROWT = (N * M) // ROWTILE        # 78 full row tiles
ROWREM = N * M - NROWT * ROWTILE  # 64

PAD_SHIFT = -512.0    # one-hot iota shift for padded edges (never matches)

_STATE: dict = {}

LAST_RESULT = None  # BassKernelResults of the most recent run (for test.py)


def _build_nc(caps):
    """caps: per-window padded edge counts (multiples of 128), same on all
    cores. Returns a compiled Bacc module."""
    stages = os.environ.get("KSTAGES", "full")  # debug bisect: A | AB | ABG | full
    T = int(sum(caps))
    assert T % 128 == 0
    TCOL = T // 128
    f32 = mybir.dt.float32
    bf16 = mybir.dt.bfloat16
    i16 = mybir.dt.int16
    nc = bacc.Bacc(
        None, target_bir_lowering=False, debug=False, num_swdge_queues=NQ
    )

    nht_d = nc.dram_tensor("nht", [F2, N * M], bf16, kind="ExternalInput")
    edge_d = nc.dram_tensor("edge", [T, H], bf16, kind="ExternalInput")
    srci_d = nc.dram_tensor("srci", [128, T // 16], i16, kind="ExternalInput")
    s0_d = nc.dram_tensor("s0", [128, TCOL * 128], bf16, kind="ExternalInput")
    wnh_d = nc.dram_tensor("wnh", [F2, OUT], bf16, kind="ExternalInput")
    bnh4_d = nc.dram_tensor("bnh4", [1, M * OUT], bf16, kind="ExternalInput")
    web_d = nc.dram_tensor("web", [128, H], bf16, kind="ExternalInput")
    beb_d = nc.dram_tensor("beb", [128, 1], f32, kind="ExternalInput")
    out_d = nc.dram_tensor("out", [N, NODE_ELEM], f32, kind="ExternalOutput")
    nhf_d = nc.dram_tensor("nhf", [N, NODE_ELEM], bf16)

    with tile.TileContext(nc) as tc:
        with (
            tc.tile_pool(name="const", bufs=1) as pc,
            tc.tile_pool(name="nht", bufs=3) as pa,
            tc.tile_pool(name="nhout", bufs=6) as pao,
            tc.tile_pool(name="edge", bufs=3) as pe,
            tc.tile_pool(name="vals", bufs=6) as pv,
            tc.tile_pool(name="onehot", bufs=6) as ps_pool,
            tc.tile_pool(name="s0", bufs=6) as ps0_pool,
            tc.tile_pool(name="flush", bufs=4) as pf,
            tc.tile_pool(name="psA", bufs=3, space="PSUM") as psA,
            tc.tile_pool(name="psW", bufs=4, space="PSUM") as psW,
        ):
            # --- constants ---
            wnh_t = pc.tile([F2, OUT], bf16)
            nc.sync.dma_start(wnh_t[:], wnh_d[:])
            bnh4_t = pc.tile([1, M * OUT], bf16)
            nc.sync.dma_start(bnh4_t[:], bnh4_d[:])
            web_t = pc.tile([128, H], bf16)
            nc.sync.dma_start(web_t[:], web_d[:])
            beb_t = pc.tile([128, 1], f32)
            nc.sync.dma_start(beb_t[:], beb_d[:])
            srci_t = pc.tile([128, T // 16], i16)
            nc.sync.dma_start(srci_t[:], srci_d[:])
            ones_t = pc.tile([1, 128], bf16)
            nc.gpsimd.memset(ones_t[:], 1.0)
            coeff_t = pc.tile([128, TCOL], f32)
            coefb_t = pc.tile([128, TCOL], bf16)
            # iota along free dim (same per partition, 4 copies)
            # PE warm-up: dependency-free matmuls so the HAM clock gate
            # releases (1.2 -> 2.4 GHz) before the real work arrives
            wrm = pc.tile([128, 512], bf16)
            nc.gpsimd.memset(wrm[:], 1.0)
            wps = psW.tile([128, 512], f32, tag="psW")
            for _ in range(16):
                nc.tensor.matmul(wps[:, :], wrm[:, :128], wrm[:], start=True, stop=True)

            # bias broadcast to all partitions via K=1 outer product
            bias_ps = psA.tile([128, M * OUT], f32, tag="psA")
            nc.tensor.matmul(bias_ps[:], ones_t[:], bnh4_t[:], start=True, stop=True)
            bias_t = pc.tile([128, M * OUT], f32)
            nc.any.tensor_copy(bias_t[:], bias_ps[:])

            # --- stage A: nh_fts projection -> nhf rows [N, 256] bf16 ---
            for t in range(NROWT + 1):
                cols = ROWTILE if t < NROWT else ROWREM
                if cols == 0:
                    break
                nht_t = pa.tile([F2, ROWTILE], bf16, tag="nht")
                nc.sync.dma_start(
                    nht_t[:, :cols], nht_d[:, t * ROWTILE : t * ROWTILE + cols]
                )
                for hh in range(0, cols, 512):
                    sub = min(512, cols - hh)
                    nodes = sub // M
                    ps = psA.tile([128, M, OUT], f32, tag="psA")
                    for m in range(M):
                        lhsT = nht_t[:, hh + m : hh + sub : M]
                        nc.tensor.matmul(
                            ps[:nodes, m, :], lhsT, wnh_t[:], start=True, stop=True
                        )
                    o_t = pao.tile([128, M * OUT], bf16, tag="nhout")
                    nc.vector.tensor_tensor(
                        o_t[:nodes, :],
                        ps[:nodes, :, :].rearrange("p m o -> p (m o)"),
                        bias_t[:nodes, :],
                        op=mybir.AluOpType.add,
                    )
                    n0 = (t * ROWTILE + hh) // M
                    nc.scalar.dma_start(nhf_d[n0 : n0 + nodes, :], o_t[:nodes, :])

            # --- stage B: coeff for the padded sorted edge stream ---
            nblk = 0 if stages == "A" else (T + BLOCK - 1) // BLOCK
            for c in range(nblk):
                e0 = c * BLOCK
                ecnt = min(BLOCK, T - e0)
                j = ecnt // 128
                et = pe.tile([128, BLOCK // 128, H], bf16, tag="edge")
                nc.sync.dma_start(
                    et[:, :j, :],
                    edge_d[e0 : e0 + ecnt].rearrange("(p jj) f -> p jj f", p=128),
                )
                web_b = bass.AP(
                    web_t[:].tensor, web_t[:].offset,
                    [web_t[:].ap[0], [0, j], web_t[:].ap[1]],
                )
                nc.vector.tensor_tensor(
                    et[:, :j, :], et[:, :j, :], web_b, op=mybir.AluOpType.mult
                )
                ccol = e0 // 128
                csl = coeff_t[:, ccol : ccol + j]
                nc.vector.tensor_reduce(
                    csl, et[:, :j, :], axis=mybir.AxisListType.X,
                    op=mybir.AluOpType.add,
                )
                nc.vector.tensor_scalar(
                    csl, csl, beb_t[:], None, mybir.AluOpType.add
                )
                nc.vector.tensor_copy(coefb_t[:, ccol : ccol + j], csl)

            # --- stage C: gather + one-hot PE scatter per window ---
            # gather groups of GCH edges, round-robined over SWDGE queues
            vtiles = []
            ngr = 0 if stages in ("A", "AB") else (T + GCH - 1) // GCH
            for g in range(ngr):
                e0 = g * GCH
                ecnt = min(GCH, T - e0)
                v = pv.tile([128, GCH // 128, NODE_ELEM], bf16, tag="vals")
                nc.gpsimd.dma_gather(
                    v[:, : ecnt // 128, :], nhf_d[:],
                    srci_t[:, e0 // 16 : (e0 + ecnt) // 16],
                    num_idxs=ecnt, num_idxs_reg=ecnt, elem_size=NODE_ELEM,
                    queue_num=g % NQ, single_packet=SP,
                )
                vtiles.append(v)

            q = 0  # global 128-edge column
            for w in range(NW if stages == "full" else 0):
                nw = min(128, N - w * 128)
                nch = caps[w] // 128
                if nch == 0:
                    z_t = pf.tile([128, NODE_ELEM], f32, tag="flush")
                    nc.any.memset(z_t[:nw, :], 0.0)
                    nc.sync.dma_start(out_d[w * 128 : w * 128 + nw, :], z_t[:nw, :])
                    continue
                pw = psW.tile([128, NODE_ELEM], f32, tag="psW")
                k = 0
                while k < nch:
                    kb = min(8, nch - k)
                    S = ps_pool.tile([128, 8, 128], bf16, tag="S")
                    s0_t = ps0_pool.tile([128, 8, 128], bf16, tag="s0")
                    nc.scalar.dma_start(
                        s0_t[:, :kb, :],
                        s0_d[:, (q + k) * 128 : (q + k + kb) * 128].rearrange(
                            "p (kb f) -> p kb f", kb=kb
                        ),
                    )
                    cf_sl = coefb_t[:, q + k : q + k + kb]
                    cf_b = bass.AP(
                        cf_sl.tensor, cf_sl.offset,
                        [cf_sl.ap[0], cf_sl.ap[1], [0, 128]],
                    )
                    nc.vector.tensor_tensor(
                        S[:, :kb, :], s0_t[:, :kb, :], cf_b, op=mybir.AluOpType.mult
                    )
                    for kk in range(kb):
                        qq = q + k + kk
                        v = vtiles[qq // (GCH // 128)]
                        vcol = qq % (GCH // 128)
                        nc.tensor.matmul(
                            pw[:, :], S[:, kk, :], v[:, vcol, :],
                            start=(k + kk == 0), stop=(k + kk == nch - 1),
                        )
                    k += kb
                q += nch
                o_t = pf.tile([128, NODE_ELEM], f32, tag="flush")
                nc.any.tensor_copy(o_t[:nw, :], pw[:nw, :])
                nc.sync.dma_start(out_d[w * 128 : w * 128 + nw, :], o_t[:nw, :])

    nc.compile()
    return nc


def _wrap_idx(idx: np.ndarray) -> np.ndarray:
    """Wrap an index stream into the [16, n/16] descriptor layout (idx i at
    [i%16, i//16]) and replicate to 128 partitions."""
    w16 = idx.reshape(-1, 16).T
    return np.tile(w16, (8, 1)).astype(np.int16)


def _interleave_rows(a: np.ndarray) -> np.ndarray:
    """Reorder edge rows so a contiguous per-partition DMA of [128, j, F]
    tiles puts logical edge l = block*BLOCK + jj*128 + p at tile[p, jj].

    DRAM row for logical l must be block*BLOCK + p*j + jj."""
    T = a.shape[0]
    out = np.empty_like(a)
    e0 = 0
    while e0 < T:
        ecnt = min(BLOCK, T - e0)
        j = ecnt // 128
        blk = a[e0 : e0 + ecnt]          # logical order [jj*128+p]
        out[e0 : e0 + ecnt] = (
            blk.reshape(j, 128, -1).transpose(1, 0, 2).reshape(ecnt, -1)
        )
        e0 += ecnt
    return out


def _prep(node_fts, hidden, edge_fts, W_nh, b_nh, W_e, b_e, edge_indices):
    """Returns (caps, in_maps)."""
    # per-core sorted edge streams
    streams = []
    counts = np.zeros((NCORES, NW), np.int64)
    for c in range(NCORES):
        b, h = divmod(c, 2)
        sl = slice(h * EH, (h + 1) * EH)
        src = np.asarray(edge_indices[b, sl, 0], np.int64)
        tgt = np.asarray(edge_indices[b, sl, 1], np.int64)
        order = np.argsort(tgt, kind="stable")
        tgt_s = tgt[order]
        counts[c] = np.bincount(tgt_s // 128, minlength=NW)
        streams.append((b, sl, order, tgt_s, src[order]))
    caps = ((counts.max(axis=0) + 127) // 128) * 128
    T = int(caps.sum())
    caps = tuple(int(x) for x in caps)

    wnh = np.ascontiguousarray(W_nh).astype(np.float32).astype(mybir.dt.np(mybir.dt.bfloat16))
    bf = wnh.dtype
    bnh4 = np.tile(np.asarray(b_nh, np.float32).reshape(1, OUT), (1, M)).astype(bf)
    web = np.tile(np.asarray(W_e, np.float32).reshape(1, H), (128, 1)).astype(bf)
    beb = np.full((128, 1), np.float32(np.asarray(b_e).reshape(-1)[0]), np.float32)

    nht_b = {}
    in_maps = []
    wstart = np.zeros(NW + 1, np.int64)
    wstart[1:] = np.cumsum(caps)
    for c in range(NCORES):
        b, sl, order, tgt_s, src_s = streams[c]
        if b not in nht_b:
            a = np.empty((F2, N * M), np.float32)
            a[:H] = node_fts[b].reshape(-1, H).T
            a[H:] = hidden[b].reshape(-1, H).T
            nht_b[b] = a.astype(bf)
        # build padded streams
        src_p = np.zeros(T, np.int64)
        tshift = np.full(T, PAD_SHIFT, np.float32)
        epos = np.full(T, -1, np.int64)  # source row in edge_fts (unsorted), -1 pad
        cnt = counts[c]
        segs = np.zeros(NW + 1, np.int64)
        segs[1:] = np.cumsum(cnt)
        for w in range(NW):
            s0, s1 = segs[w], segs[w + 1]
            d0 = wstart[w]
            n = s1 - s0
            src_p[d0 : d0 + n] = src_s[s0:s1]
            tshift[d0 : d0 + n] = tgt_s[s0:s1] - 128 * w
            epos[d0 : d0 + n] = order[s0:s1]
        # edge features in padded-sorted order (pads zero), interleaved
        ef = np.zeros((T, H), np.float32)
        valid = epos >= 0
        ef[valid] = np.asarray(edge_fts[b, sl], np.float32)[epos[valid]]
        ef = _interleave_rows(ef.astype(bf))
        tsc = tshift.reshape(-1, 128).T  # [128, TCOL]
        s0u = np.zeros((128, tsc.shape[1] * 128), np.uint16)
        vmask = (tsc >= 0) & (tsc < 128)
        pp, qq = np.nonzero(vmask)
        s0u[pp, qq * 128 + tsc[pp, qq].astype(np.int64)] = 0x3F80
        in_maps.append({
            "nht": nht_b[b],
            "edge": ef,
            "srci": _wrap_idx(src_p.astype(np.int16)),
            "s0": s0u.view(bf),
            "wnh": wnh,
            "bnh4": bnh4,
            "web": web,
            "beb": beb,
        })
    return caps, in_maps


def _get_nc(caps):
    key = ("nc", caps)
    if key not in _STATE:
        _STATE[key] = _build_nc(caps)
    return _STATE[key]


def kernel(node_fts, hidden, edge_fts, W_nh, b_nh, W_e, b_e, edge_indices):
    global LAST_RESULT
    caps, in_maps = _prep(
        node_fts, hidden, edge_fts, W_nh, b_nh, W_e, b_e, edge_indices
    )
    nc = _get_nc(caps)
    res = run_bass_kernel_spmd(nc, in_maps, core_ids=list(range(NCORES)))
    LAST_RESULT = res
    out = np.empty((B, N, M, OUT), np.float32)
    for b in range(B):
        acc = res.results[2 * b]["out"] + res.results[2 * b + 1]["out"]
        out[b] = acc.reshape(N, M, OUT)
    return out


def run_core_sim(core_id, caps, in_map, zero_out=True):
    """Simulate a single core's program on its in_map via CoreSim (test use)."""
    from concourse.bass_interp import CoreSim

    nc = _get_nc(caps)
    sim = CoreSim(nc, trace=False)
    for k, v in in_map.items():
        sim.tensor(k)[:] = v
    if zero_out:
        sim.tensor("out")[:] = 0
    sim.simulate()
    return np.array(sim.tensor("out"))
